# revision 31
# baseline (speedup 1.0000x reference)
"""Multi-head attention TRN2 kernel (Bass/Tile), 8-core tensor-parallel.

Sharding: core c -> batch b=c//4, head group g=c%4 (4 heads = 256 features).
Host pre-transposes x and weight slices to bf16; device computes qT/kT
(features x tokens), v natural (tokens x features, ones-augmented), causal
softmax via unnormalized exp + sum-row from ones-column of V, normalization
via PE row-broadcast of a fast reciprocal, and the output projection
partial (bf16). Host sums the 4 partials per batch in f32 and adds bias.

v2 vs baseline:
- bf16 operands everywhere (PSUM stays f32): halves DVE copy cost, DMA
  traffic, LDWEIGHTS time (FWL), keeps matmul at 1 col/cycle at any N.
- QK matmuls row-packed per head pair (contraction=64) via tile_position
  rows 0/64 -> both heads' scores in one PE pass.
- Diagonal 512-block sliced at 128 granularity (packed PSUM tiles
  [512|384] + [256|128]): exp/AV/QK only touch the causal 53% instead of
  62.5%, no zeroing pass; per-chunk triangular mask-mult on GpSimd.
- reciprocal_approx_fast for softmax sums (was 3.4us full-precision DVE
  reciprocal); bitcast instead of copy for the f32r broadcast operand.
- bf16 output partials, host accumulates in f32.
"""

import numpy as np

B, S, D = 2, 2048, 1024
H, HD = 16, 64
NCORES = 8
HPC = 4              # heads per core
FPC = HPC * HD       # 256 features per core
NF = FPC // 128      # 2 feature tiles of 128
KC = D // 128        # 8 contraction chunks
NTS = S // 512       # 4 token slices == q-tiles
NTT = S // 128       # 16 token tiles of 128
SCALE = 1.0 / 8.0    # 1/sqrt(HD)

_CACHE = {}


def _legalize_waits(nc, mybir, max_waits=1):
    """Walrus codegen allows only 1 sync-wait slot on most TPB instructions.
    Hoist extra waits into same-engine NoOps inserted just before."""
    n_fixed = 0
    for _, bb_wrap in nc.bb_map.items():
        bb = bb_wrap.bb
        out = []
        changed = False
        for inst in list(bb.instructions):
            si = inst.sync_info
            if si is not None and si.on_wait and len(si.on_wait) > max_waits:
                for w in list(si.on_wait[:-max_waits]):
                    nop = mybir.InstNoOp(
                        name=f"I-lw-{nc.next_id()}", engine=inst.engine,
                        ins=[], outs=[],
                        sync_info=mybir.SyncInfo(on_wait=[w], on_update=[]),
                    )
                    nop.text_hint = "dep"
                    out.append(nop)
                si.on_wait = list(si.on_wait[-max_waits:])
                n_fixed += 1
                changed = True
            out.append(inst)
        if changed:
            bb.instructions = out
    return n_fixed


def _build(legalize=True):
    import concourse.bass as bass
    import concourse.mybir as mybir
    from concourse.tile import TileContext
    from concourse.masks import make_upper_triangular

    F32 = mybir.dt.float32
    F32R = mybir.dt.float32r
    BF16 = mybir.dt.bfloat16
    EXP = mybir.ActivationFunctionType.Exp
    LN = mybir.ActivationFunctionType.Ln
    MUL = mybir.AluOpType.mult

    nc = bass.Bass()
    # host-reshaped: partition-major chunks packed along the free axis so
    # each tensor loads in one wide DMA.
    xT = nc.dram_tensor("xT", [128, KC * S], BF16, kind="ExternalInput")
    wqT = nc.dram_tensor("wqT", [128, KC * FPC], BF16, kind="ExternalInput")
    wkT = nc.dram_tensor("wkT", [128, KC * FPC], BF16, kind="ExternalInput")
    wvT = nc.dram_tensor("wvT", [128, KC * FPC], BF16, kind="ExternalInput")
    woT = nc.dram_tensor("woT", [128, NF * D], BF16, kind="ExternalInput")
    outp = nc.dram_tensor("outp", [S, D], BF16, kind="ExternalOutput")

    with TileContext(nc) as tc:
        with (
            tc.tile_pool(name="res", bufs=1) as res,
            tc.tile_pool(name="xp", bufs=2) as xp,
            tc.tile_pool(name="wk", bufs=4) as wkp,
            tc.tile_pool(name="osb", bufs=6) as osb,
            tc.tile_pool(name="pjps", bufs=1, space="PSUM") as pjps,
            tc.tile_pool(name="pops", bufs=1, space="PSUM") as pops,
            tc.tile_pool(name="qkps", bufs=2, space="PSUM") as qkps,
            tc.tile_pool(name="avps", bufs=2, space="PSUM") as avps,
        ):
            # ---- resident tensors -------------------------------------
            qT = [res.tile([128, S], BF16, name=f"qT{f}", tag=f"qT{f}")
                  for f in range(NF)]
            kT = [res.tile([128, S], BF16, name=f"kT{f}", tag=f"kT{f}")
                  for f in range(NF)]
            ctxT = [res.tile([128, S], BF16, name=f"ctxT{f}", tag=f"ctxT{f}")
                    for f in range(NF)]
            v_sb = [res.tile([128, 4 * 65], BF16, name=f"v{ck}", tag=f"v{ck}")
                    for ck in range(NTT)]
            # x chunks for the first token slice: loaded before weights so
            # the first projection chain starts as early as possible.
            def load_x(ts):
                # per-chunk DMAs (contiguous slice-major layout) so the
                # first projection group starts on chunk 0's arrival.
                xbig = xp.tile([128, KC * 512], BF16, name="xbig",
                               tag="xbig")
                base = ts * KC * 512
                for e in range(KC):
                    nc.sync.dma_start(
                        out=xbig[:, e * 512:(e + 1) * 512],
                        in_=xT[:, base + e * 512:base + (e + 1) * 512])
                return [xbig[:, e * 512:(e + 1) * 512] for e in range(KC)]

            w_t = {}

            def load_w(nm, dram):
                t = wkp.tile([128, KC * FPC], BF16, name=f"w{nm}",
                             tag=f"w{nm}", bufs=1)
                for e in range(KC):
                    nc.scalar.dma_start(
                        out=t[:, e * FPC:(e + 1) * FPC],
                        in_=dram[:, e * FPC:(e + 1) * FPC])
                    w_t[nm, e] = t[:, e * FPC:(e + 1) * FPC]

            # Q weights first so the first projection group can start as
            # soon as x arrives; K/V weights stream in behind it.
            load_w("q", wqT)
            xch0 = load_x(0)
            load_w("k", wkT)
            load_w("v", wvT)

            # constants
            ones_f = res.tile([128, 4], BF16)
            nc.gpsimd.memset(ones_f, 1.0)
            ones_row_f = res.tile([1, 128], F32)
            nc.gpsimd.memset(ones_row_f, 1.0)
            # selector for the per-head-pair reciprocal broadcast: head
            # sums land on partitions 0 / 64 of s2; sel2[r, p] = 1 iff
            # (r == 0, p < 64) or (r == 64, p >= 64), so sel2.T @ rcp2
            # routes each head's reciprocal row onto its partition range.
            sel_f = res.tile([128, 128], F32)
            nc.gpsimd.memset(sel_f, 0.0)
            nc.vector.tensor_copy(sel_f[0:1, 0:64], ones_row_f[0:1, 0:64])
            nc.vector.tensor_copy(sel_f[64:65, 64:128],
                                  ones_row_f[0:1, 0:64])
            sel_r = res.tile([128, 128], F32R)
            nc.vector.tensor_copy(sel_r, sel_f)
            # scratch for the ln/exp reciprocal; rows other than 0/64 stay
            # 1.0 forever so ln/exp of them is finite and sel2 zeroes them.
            s2 = res.tile([128, 512], F32)
            nc.gpsimd.memset(s2, 1.0)
            ln2 = res.tile([128, 512], F32)
            rcp2 = res.tile([128, 512], F32R)
            mask_f = res.tile([128, 128], F32)
            make_upper_triangular(nc, mask_f, val=1.0, diag=True)
            mask_b = res.tile([128, 128], BF16)
            nc.vector.tensor_copy(mask_b, mask_f)
            for ck in range(NTT):
                v5 = v_sb[ck].rearrange("p (g c) -> p g c", c=65)
                nc.vector.tensor_copy(v5[:, 0:4, 64], ones_f)

            wo_big = res.tile([128, NF * D], BF16, name="wo", tag="wo")
            nc.scalar.dma_start(out=wo_big, in_=woT[:, :])
            woT_sb = [wo_big[:, ic * D:(ic + 1) * D] for ic in range(NF)]

            # ---- pipelined slices -------------------------------------
            for ts in range(NTS):
                xch = xch0 if ts == 0 else load_x(ts)

                # projections for this slice
                for nm, dst in (("q", qT), ("k", kT)):
                    for f in range(NF):
                        ps = pjps.tile([128, 512], F32, name="pqk", tag="sm")
                        for e in range(KC):
                            nc.tensor.matmul(
                                ps, w_t[nm, e][:, f * 128:(f + 1) * 128],
                                xch[e], start=(e == 0), stop=(e == KC - 1))
                        nc.vector.tensor_copy(
                            dst[f][:, ts * 512:(ts + 1) * 512], ps)
                for tt in range(4):
                    ck = ts * 4 + tt
                    ps = pjps.tile([128, FPC], F32, name="pv", tag="sm")
                    for e in range(KC):
                        nc.tensor.matmul(
                            ps, xch[e][:, tt * 128:(tt + 1) * 128],
                            w_t["v", e], start=(e == 0), stop=(e == KC - 1))
                    v5 = v_sb[ck].rearrange("p (g c) -> p g c", c=65)
                    ps4 = ps.rearrange("p (g c) -> p g c", c=64)
                    nc.vector.tensor_copy(v5[:, 0:4, 0:64], ps4)

                # attention for q-tile j == ts, all 4 local heads.
                # Head pair (rows 0:64 / 64:128 of the f-tile) runs QK
                # row-packed on the PE (tile_position auto from
                # base_partition 0/64).
                j = ts
                sq0 = 512 * j
                for f in range(NF):
                    av = [avps.tile([128, 512], F32, name=f"av{hl}",
                                    tag="av") for hl in range(2)]

                    # one [128, <=1024] PSUM tile per key-chunk holds BOTH
                    # heads' scores (head hl at column offset h1off*hl);
                    # the pair of QK matmuls row-packs on the PE
                    # (contraction 64, tile_position rows 0/64), one exp
                    # covers both heads.
                    # full (below-diagonal) chunks: width 512 per head
                    for ci in range(4 * j):
                        qk = qkps.tile([128, 1024], F32, name="qk", tag="qk")
                        for hl in range(2):
                            r0 = 64 * hl
                            nc.tensor.matmul(
                                qk[:, 512 * hl:512 * hl + 512],
                                kT[f][r0:r0 + 64, ci * 128:(ci + 1) * 128],
                                qT[f][r0:r0 + 64, sq0:sq0 + 512],
                                start=True, stop=True)
                        ag = osb.tile([128, 1024], BF16, name="ag", tag="ag")
                        nc.scalar.activation(ag, qk, EXP, scale=SCALE)
                        for hl in range(2):
                            hh = 2 * f + hl
                            nc.tensor.matmul(
                                av[hl][0:65, :],
                                v_sb[ci][:, 65 * hh:65 * hh + 65],
                                ag[:, 512 * hl:512 * hl + 512],
                                start=(ci == 0), stop=False)

                    # diagonal 512-block: 4 chunks at offsets 0/128/256/384,
                    # causally sliced to width 512-d per head; head blocks
                    # packed so each matmul stays within one PSUM bank.
                    for dd in range(4):
                        d = 128 * dd
                        w = 512 - d
                        ci = 4 * j + dd
                        # head hl's block at column 512*hl; for dd>0 the
                        # heads sit in separate PSUM banks (h1 at 512) so
                        # the two row-packed matmuls never share a bank.
                        qk = qkps.tile([128, 1024], F32, name="qkd",
                                       tag="qk")
                        for hl in range(2):
                            r0 = 64 * hl
                            nc.tensor.matmul(
                                qk[:, 512 * hl:512 * hl + w],
                                kT[f][r0:r0 + 64, ci * 128:(ci + 1) * 128],
                                qT[f][r0:r0 + 64, sq0 + d:sq0 + 512],
                                start=True, stop=True)
                        ag = osb.tile([128, 1024], BF16, name="agd",
                                      tag="ag")
                        if dd == 0:
                            nc.scalar.activation(ag, qk, EXP, scale=SCALE)
                        else:
                            # one strided activation over both head blocks
                            # (the [w:512] gap per block is uninit PSUM)
                            qk2 = qk.rearrange("p (h c) -> p h c", c=512)
                            ag2 = ag.rearrange("p (h c) -> p h c", c=512)
                            nc.scalar.activation(ag2[:, :, 0:w],
                                                 qk2[:, :, 0:w],
                                                 EXP, scale=SCALE)
                        for hl in range(2):
                            c0 = 512 * hl
                            nc.gpsimd.tensor_tensor(
                                ag[:, c0:c0 + 128], ag[:, c0:c0 + 128],
                                mask_b, MUL)
                        for hl in range(2):
                            hh = 2 * f + hl
                            c0 = 512 * hl
                            nc.tensor.matmul(
                                av[hl][0:65, d:512],
                                v_sb[ci][:, 65 * hh:65 * hh + 65],
                                ag[:, c0:c0 + w],
                                start=(ci == 0), stop=(dd == 3))

                    # normalize: ctxT = av[0:64] * rowbcast(1/av[64]).
                    # 1/s computed as exp(-ln(s)) on ACT (same table set as
                    # the softmax exp), batched over the head pair; one
                    # selector matmul broadcasts each head's row to its
                    # 64-partition range.
                    for hl in range(2):
                        nc.vector.tensor_copy(s2[64 * hl:64 * hl + 1, :],
                                              av[hl][64:65, :])
                    nc.scalar.activation(ln2, s2, LN)
                    nc.scalar.activation(rcp2, ln2, EXP, scale=-1.0)
                    bc = pops.tile([128, 512], F32, name="bc", tag="po")
                    nc.tensor.matmul(bc, sel_r, rcp2, start=True, stop=True)
                    bcs = osb.tile([128, 512], F32, name="bcs", tag="bcs")
                    nc.vector.tensor_copy(bcs, bc)
                    for hl in range(2):
                        r0 = 64 * hl
                        nc.vector.tensor_tensor(
                            ctxT[f][r0:r0 + 64, sq0:sq0 + 512],
                            av[hl][0:64, :], bcs[r0:r0 + 64, :], MUL)

                # output projection for this slice's token tiles
                for tt in range(4 * ts, 4 * ts + 4):
                    so = osb.tile([128, 1024], BF16, name="so", tag="so")
                    for os_ in range(2):
                        po = pops.tile([128, 512], F32, name="po", tag="po")
                        for ic in range(NF):
                            nc.tensor.matmul(
                                po,
                                ctxT[ic][:, tt * 128:(tt + 1) * 128],
                                woT_sb[ic][:, os_ * 512:(os_ + 1) * 512],
                                start=(ic == 0), stop=(ic == NF - 1))
                        nc.vector.tensor_copy(
                            so[:, os_ * 512:(os_ + 1) * 512], po)
                    nc.sync.dma_start(
                        out=outp[tt * 128:(tt + 1) * 128, :], in_=so)

    if legalize:
        _legalize_waits(nc, mybir)
    return nc


def _chunk_major(a):
    """[n*128, F] -> [128, n*F]: partition-major 128-row chunks packed
    along the free axis (one wide DMA per tensor on device)."""
    n = a.shape[0] // 128
    return np.ascontiguousarray(
        a.reshape(n, 128, a.shape[1]).transpose(1, 0, 2).reshape(128, -1))


def _prep_inputs(in_data, Wq, Wk, Wv, Wo):
    import ml_dtypes

    bf16 = ml_dtypes.bfloat16
    # x: slice-major [ts][chunk-major 128 x 4096] so each per-slice load is
    # one contiguous 8KB-per-partition DMA.
    xTb = []
    for b in range(B):
        cm = _chunk_major(in_data[b].T)              # [128, KC*S]
        cm = cm.reshape(128, KC, NTS, 512).transpose(0, 2, 1, 3)
        xTb.append(np.ascontiguousarray(cm.reshape(128, -1)).astype(bf16))
    in_maps = []
    for c in range(NCORES):
        b, g = c // 4, c % 4
        sl = slice(g * FPC, (g + 1) * FPC)
        in_maps.append({
            "xT": xTb[b],
            "wqT": _chunk_major(Wq[sl, :].T).astype(bf16),
            "wkT": _chunk_major(Wk[sl, :].T).astype(bf16),
            "wvT": _chunk_major(Wv[sl, :].T).astype(bf16),
            "woT": _chunk_major(Wo[:, sl].T).astype(bf16),
        })
    return in_maps


def run(inputs, trace=False):
    from concourse.bass_utils import run_bass_kernel_spmd

    in_data = np.asarray(inputs["in_data"], dtype=np.float32)
    Wq = np.asarray(inputs["Wq"], dtype=np.float32)
    Wk = np.asarray(inputs["Wk"], dtype=np.float32)
    Wv = np.asarray(inputs["Wv"], dtype=np.float32)
    Wo = np.asarray(inputs["Wo"], dtype=np.float32)
    bo = np.asarray(inputs["bo"], dtype=np.float32)

    if "nc" not in _CACHE:
        _CACHE["nc"] = _build()
    nc = _CACHE["nc"]

    in_maps = _prep_inputs(in_data, Wq, Wk, Wv, Wo)
    kw = {}
    if trace:
        kw = dict(trace=True, trace_cores=list(range(NCORES)))
    res = run_bass_kernel_spmd(nc, in_maps, core_ids=list(range(NCORES)), **kw)

    out = np.zeros((B, S, D), dtype=np.float32)
    for c in range(NCORES):
        out[c // 4] += res.results[c]["outp"].astype(np.float32)
    out += bo[None, None, :]
    return out, res


def kernel(**inputs) -> np.ndarray:
    out, _ = run(inputs)
    return out
